# revision 32
# baseline (speedup 1.0000x reference)
"""Multi-head attention (B=2, S=2048, D=768, H=12, Dh=64) on 8 TRN2 cores.

Sharding: core = (batch b = core//4, head-group g = core%4 of 3 heads).
Each core computes its 3 heads' attention for its batch and a partial
output projection [S, 768]; host sums the 4 group-partials per batch and
adds b_proj.

v2: fully software-pipelined single round structure.  All PE inputs are
f16 (half the DMA bytes and SBUF of f32).  Per (qt, key-chunk) "round":
two quadrant-paired score matmuls -> one exp on ACT -> two context
matmuls.  Rounds rotate through a 3-slot PSUM ring and a 6-slot SBUF pt
pool so no round has a dependency on the previous round's consumers
(the v1 kernel serialized on a pt-tile WAR and kept the PE at its
1.2 GHz mid P-state).  QKV streams, V, normalize and the projection are
interleaved into round "post" slots to keep both PE and ACT busy.
Normalize: in-place reciprocal_approx_fast on the Z row + PE ones-
broadcast into a ring slot + DVE multiply (v1 used a 3.3us DVE
reciprocal and a DRAM broadcast round-trip per head/qt).
"""

import numpy as np

B = 2
S = 2048
D = 768
NH = 12
DH = 64
NCORES = 8
P = 128
KCH = D // P          # 6 k-chunks for the QKV projection
NQT = S // 512        # 4 query tiles of 512
NKC = S // P          # 16 key chunks of 128

_CACHE = {}


def _build():
    import concourse.mybir as mybir
    import concourse.tile as tile
    from concourse import bacc

    F32 = mybir.dt.float32
    F32R = mybir.dt.float32r
    F16 = mybir.dt.float16
    EXP = mybir.ActivationFunctionType.Exp
    LN = mybir.ActivationFunctionType.Ln

    nc = bacc.Bacc(target_bir_lowering=False, debug=False)

    xt_d = nc.dram_tensor("xt", [D, S], F16, kind="ExternalInput")
    wq01_d = nc.dram_tensor("wq01", [D, P], F16, kind="ExternalInput")
    wq2d_d = nc.dram_tensor("wq2d", [D, P], F16, kind="ExternalInput")
    wk01_d = nc.dram_tensor("wk01", [D, P], F16, kind="ExternalInput")
    wk2d_d = nc.dram_tensor("wk2d", [D, P], F16, kind="ExternalInput")
    wv_d = nc.dram_tensor("wv", [D, 3 * DH], F16, kind="ExternalInput")
    bq01_d = nc.dram_tensor("bq01", [P, 1], F32, kind="ExternalInput")
    bq2d_d = nc.dram_tensor("bq2d", [P, 1], F32, kind="ExternalInput")
    bk01_d = nc.dram_tensor("bk01", [P, 1], F32, kind="ExternalInput")
    bk2d_d = nc.dram_tensor("bk2d", [P, 1], F32, kind="ExternalInput")
    bv_d = nc.dram_tensor("bv", [1, 3 * DH], F32, kind="ExternalInput")
    wp_d = [nc.dram_tensor(f"wp{h}", [DH, D], F16, kind="ExternalInput")
            for h in range(3)]
    ones_d = nc.dram_tensor("ones1", [1, 1], F16, kind="ExternalInput")
    out_d = nc.dram_tensor("out", [S, D], F32, kind="ExternalOutput")

    with tile.TileContext(nc) as tc:
        with (
            tc.sbuf_pool(name="pw", bufs=1) as pw,
            tc.sbuf_pool(name="px", bufs=1) as px,
            tc.sbuf_pool(name="pqk", bufs=1) as pqk,
            tc.sbuf_pool(name="pv", bufs=1) as pv,
            tc.sbuf_pool(name="ppt", bufs=1) as ppt,
            tc.sbuf_pool(name="pctn", bufs=1) as pctn,
            tc.sbuf_pool(name="pz", bufs=1) as pz,
            tc.psum_pool(name="ps", bufs=1) as ps,
            tc.sbuf_pool(name="pout", bufs=3) as pout,
        ):
            # ---- weight / bias / input loads, spread across DMA queues ----
            wq01 = pw.tile([P, KCH, P], F16)
            wq2d = pw.tile([P, KCH, P], F16)
            wk01 = pw.tile([P, KCH, P], F16)
            wk2d = pw.tile([P, KCH, P], F16)
            wv = pw.tile([P, KCH, 3 * DH], F16)
            bq01 = pw.tile([P, 1], F32)
            bq2d = pw.tile([P, 1], F32)
            bk01 = pw.tile([P, 1], F32)
            bk2d = pw.tile([P, 1], F32)
            bvb = pw.tile([P, 3 * DH], F32)
            wp = [pw.tile([DH, D], F16, name=f"wp{h}") for h in range(3)]
            onesr = pw.tile([P, DH], F16)
            xt = px.tile([P, KCH, S], F16)
            xtr = xt_d.ap().rearrange("(c p) s -> c p s", p=P)

            # sync queue: k01 path first (first consumer), then xt c0/c1
            nc.sync.dma_start(out=bk01, in_=bk01_d.ap())
            nc.sync.dma_start(
                out=wk01, in_=wk01_d.ap().rearrange("(c p) m -> p c m", p=P))
            nc.sync.dma_start(out=xt[:, 0, :], in_=xtr[0])
            nc.sync.dma_start(out=xt[:, 1, :], in_=xtr[1])
            for h in range(3):
                nc.sync.dma_start(out=wp[h], in_=wp_d[h].ap())
            # gpsimd (Pool) queue
            nc.gpsimd.dma_start(out=bk2d, in_=bk2d_d.ap())
            nc.gpsimd.dma_start(
                out=wk2d, in_=wk2d_d.ap().rearrange("(c p) m -> p c m", p=P))
            nc.gpsimd.dma_start(out=xt[:, 2, :], in_=xtr[2])
            nc.gpsimd.dma_start(out=xt[:, 3, :], in_=xtr[3])
            nc.gpsimd.dma_start(out=bvb, in_=bv_d.ap().to_broadcast([P, 3 * DH]))
            # scalar (ACT) queue — startup only, ACT is otherwise idle here
            nc.scalar.dma_start(out=bq01, in_=bq01_d.ap())
            nc.scalar.dma_start(
                out=wq01, in_=wq01_d.ap().rearrange("(c p) m -> p c m", p=P))
            nc.scalar.dma_start(out=xt[:, 4, :], in_=xtr[4])
            nc.scalar.dma_start(out=xt[:, 5, :], in_=xtr[5])
            nc.scalar.dma_start(
                out=wq2d, in_=wq2d_d.ap().rearrange("(c p) m -> p c m", p=P))
            nc.scalar.dma_start(
                out=wv, in_=wv_d.ap().rearrange("(c p) m -> p c m", p=P))
            nc.scalar.dma_start(out=bq2d, in_=bq2d_d.ap())

            # ---- persistent SBUF tensors ----
            k01 = pqk.tile([P, S], F16)
            k2d = pqk.tile([P, S], F16)
            q01 = pqk.tile([P, S], F16)
            q2d = pqk.tile([P, S], F16)
            v3 = pv.tile([P, NKC, 3, DH + 1], F16)
            ctn = [pctn.tile([DH, NQT, 512], F16, name=f"ctn{h}")
                   for h in range(3)]
            warm = pz.tile([P, 1], F32, name="warm")
            warm16 = pz.tile([P, 1], F16, name="warm16")
            nc.vector.memset(onesr, 1.0)

            for h in range(3):
                nc.sync.dma_start(
                    out=v3[:, :, h, DH:DH + 1],
                    in_=ones_d.ap().to_broadcast([P, NKC, 1]))

            def ring(name):
                return ps.tile([P, 2, 512], F32, tag="s", bufs=3, name=name,
                               uniquify=True)

            # accumulate in the order xt chunks land from the 3 DMA queues
            CORDER = [0, 2, 4, 1, 3, 5]

            def stream_chunk(dst, w, bias, qt, name):
                # one 512-wide slab of a QKV output stream
                acc = ring(name)
                for ci, c in enumerate(CORDER):
                    nc.tensor.matmul(
                        acc[:, 0, :], w[:, c, :],
                        xt[:, c, qt * 512:(qt + 1) * 512],
                        start=(ci == 0), stop=(ci == KCH - 1))
                nc.vector.tensor_scalar_add(
                    out=dst[:, qt * 512:(qt + 1) * 512], in0=acc[:, 0, :],
                    scalar1=bias)

            def v_group(sc):
                acc = ring(f"v{sc}")
                for c in range(KCH):
                    nc.tensor.matmul(
                        acc[:, 0, 0:3 * DH], xt[:, c, sc * P:(sc + 1) * P],
                        wv[:, c, :], start=(c == 0), stop=(c == KCH - 1))
                for h in range(3):
                    nc.vector.tensor_add(
                        v3[:, sc, h, 0:DH], acc[:, 0, h * DH:(h + 1) * DH],
                        bvb[:, h * DH:(h + 1) * DH])

            # ---- round definitions ----
            # one round = 2 paired score matmuls -> exp -> 2 ctx matmuls.
            # p01 rounds: heads 0,1 at key chunk c.  h2 rounds: head 2 at
            # key chunks (2rr, 2rr+1) via the duplicated k2d/q2d halves.
            ct = {}

            def make_p01(qt, c):
                def scores(slot):
                    q0 = qt * 512
                    nc.tensor.matmul(
                        slot[:, 0, :],
                        k01[0:DH, c * P:(c + 1) * P],
                        q01[0:DH, q0:q0 + 512], start=True, stop=True)
                    nc.tensor.matmul(
                        slot[:, 1, :],
                        k01[DH:P, c * P:(c + 1) * P],
                        q01[DH:P, q0:q0 + 512], start=True, stop=True)

                def ctx(pt):
                    if c == 0:
                        ct[(qt, 0)] = ps.tile([DH + 1, 512], F32, tag="ct",
                                              bufs=2, name=f"ct0_{qt}",
                                              uniquify=True)
                        ct[(qt, 1)] = ps.tile([DH + 1, 512], F32, tag="ct",
                                              bufs=2, name=f"ct1_{qt}",
                                              uniquify=True)
                    for h in range(2):
                        nc.tensor.matmul(
                            ct[(qt, h)], v3[:, c, h, :],
                            pt[:, h, :], start=(c == 0), stop=(c == NKC - 1))
                return scores, ctx

            def make_h2(qt, rr):
                c = 2 * rr

                def scores(slot):
                    q0 = qt * 512
                    nc.tensor.matmul(
                        slot[:, 0, :],
                        k2d[0:DH, c * P:(c + 1) * P],
                        q2d[0:DH, q0:q0 + 512], start=True, stop=True)
                    nc.tensor.matmul(
                        slot[:, 1, :],
                        k2d[DH:P, (c + 1) * P:(c + 2) * P],
                        q2d[DH:P, q0:q0 + 512], start=True, stop=True)

                def ctx(pt):
                    if rr == 0:
                        ct[(qt, 2)] = ps.tile([DH + 1, 512], F32, tag="ct",
                                              bufs=2, name=f"ct2_{qt}",
                                              uniquify=True)
                    for j in range(2):
                        nc.tensor.matmul(
                            ct[(qt, 2)], v3[:, c + j, 2, :],
                            pt[:, j, :], start=(rr == 0 and j == 0),
                            stop=(rr == NKC // 2 - 1 and j == 1))
                return scores, ctx

            def normalize(qt, heads):
                # All DVE copies first (each frees its ct PSUM slot).
                # 1/Z = exp(-ln Z) in-place on the Z row via two ACT ops
                # (ln and exp share one ACT table set; ACT has slack and
                # this avoids the 3.3us DVE reciprocal on the PE's
                # critical path).  Then PE ones-broadcast into a ring
                # slot and DVE multiply into ctn[h].  No DMA involved.
                ctus = []
                for h in heads:
                    ctu = pz.tile([DH + 1, 512], F32R, tag="ctu", bufs=3,
                                  name=f"cu{h}{qt}", uniquify=True)
                    nc.vector.tensor_copy(ctu, ct[(qt, h)])
                    ctus.append(ctu)
                bc = ring(f"bc{qt}{heads[0]}")
                for j, h in enumerate(heads):
                    ctu = ctus[j]
                    zrow = ctu[DH:DH + 1, :].bitcast(F32)
                    nc.scalar.activation(zrow, zrow, LN)
                    # 1/Z lands in f16 so the broadcast matmul runs in
                    # f16 (avoids the BIR verifier's f32r rounding rule)
                    zrec = pz.tile([DH + 1, 512], F16, tag="zr", bufs=3,
                                   name=f"zr{h}{qt}", uniquify=True)
                    nc.scalar.activation(zrec[DH:DH + 1, :], zrow, EXP,
                                         scale=-1.0)
                    nc.tensor.matmul(
                        bc[0:DH, j, :], onesr[DH:DH + 1, 0:DH],
                        zrec[DH:DH + 1, :], start=True, stop=True)
                    nc.vector.tensor_mul(
                        ctn[h][:, qt, :], ctu[0:DH, :].bitcast(F32),
                        bc[0:DH, j, :])

            def proj_st(qt, st):
                pp = ring(f"pp{qt}{st}")
                for r0, r1, reg in ((0, 512, 0), (512, D, 1)):
                    for h in range(3):
                        nc.tensor.matmul(
                            pp[:, reg, 0:r1 - r0],
                            ctn[h][:, qt, st * P:(st + 1) * P],
                            wp[h][:, r0:r1], start=(h == 0), stop=(h == 2))
                stage = pout.tile([P, D], F32, tag="stage",
                                  name=f"st{qt}{st}", uniquify=True)
                nc.vector.tensor_copy(stage[:, 0:512], pp[:, 0, :])
                nc.vector.tensor_copy(stage[:, 512:D], pp[:, 1, 0:256])
                r0 = qt * 512 + st * P
                nc.gpsimd.dma_start(out=out_d.ap()[r0:r0 + P, :], in_=stage)

            # ---- build the global round list with post-work ----
            rounds = []
            posts = {}
            for qt in range(NQT):
                base = qt * 24
                for c in range(NKC):
                    rounds.append(make_p01(qt, c))
                for rr in range(NKC // 2):
                    rounds.append(make_h2(qt, rr))
                posts[base + 15] = [lambda qt=qt: normalize(qt, [0, 1])]
                posts[base + 23] = [lambda qt=qt: normalize(qt, [2])]
                if qt < NQT - 1:
                    # spread proj st-chunks well into the next qt's rounds
                    # so they never wait on the normalize chain from the
                    # in-order PE stream
                    for st in range(4):
                        posts.setdefault(base + 33 + 4 * st, []).append(
                            lambda qt=qt, st=st: proj_st(qt, st))
            # qt0 fill-in: V groups and remaining QKV streams
            posts[0] = [lambda: v_group(2), lambda: v_group(3)]
            posts[1] = [lambda: stream_chunk(q2d, wq2d, bq2d, 0, "q2d0")]
            posts[2] = [lambda: v_group(4), lambda: v_group(5)]
            posts[3] = [lambda: stream_chunk(k2d, wk2d, bk2d, 0, "k2d0")]
            posts[4] = [lambda: v_group(6), lambda: v_group(7)]
            posts[5] = [lambda: stream_chunk(k2d, wk2d, bk2d, 1, "k2d1")]
            posts[6] = [lambda: v_group(8), lambda: v_group(9)]
            posts[7] = [lambda: stream_chunk(k2d, wk2d, bk2d, 2, "k2d2")]
            posts[8] = [lambda: v_group(10), lambda: v_group(11)]
            posts[9] = [lambda: stream_chunk(k2d, wk2d, bk2d, 3, "k2d3")]
            posts[10] = [lambda: v_group(12), lambda: v_group(13)]
            posts[11] = [lambda: stream_chunk(q01, wq01, bq01, 1, "q011")]
            posts[12] = [lambda: v_group(14), lambda: v_group(15)]
            posts[13] = [lambda: stream_chunk(q2d, wq2d, bq2d, 1, "q2d1")]
            # later qt fill-in: next qt's q streams
            for qt in (1, 2):
                posts.setdefault(qt * 24 + 3, []).append(
                    lambda qt=qt: stream_chunk(q01, wq01, bq01, qt + 1,
                                               f"q01{qt + 1}"))
                posts.setdefault(qt * 24 + 9, []).append(
                    lambda qt=qt: stream_chunk(q2d, wq2d, bq2d, qt + 1,
                                               f"q2d{qt + 1}"))

            # k01 slabs 1-3 land mid-stream, just before the first scores
            # round that reads them (round 4s reads slab s)
            pres = {2: [lambda: stream_chunk(k01, wk01, bk01, 1, "k011")],
                    6: [lambda: stream_chunk(k01, wk01, bk01, 2, "k012")],
                    10: [lambda: stream_chunk(k01, wk01, bk01, 3, "k013")]}

            # ---- prologue: only what rounds 0-3 need ----
            stream_chunk(k01, wk01, bk01, 0, "k010")
            stream_chunk(q01, wq01, bq01, 0, "q010")
            v_group(0)
            v_group(1)
            # warm the ACT ln/exp table so round 0 doesn't pay the load
            nc.vector.memset(warm, 1.0)
            nc.scalar.activation(warm16, warm, EXP, scale=0.125)
            nc.scalar.activation(warm16, warm, LN)

            # ---- software-pipelined main loop ----
            NR = len(rounds)
            slots = {}
            pts = {}
            for i in range(NR + 2):
                for fn in pres.get(i, ()):
                    fn()
                if i < NR:
                    slots[i] = ring(f"r{i}")
                    rounds[i][0](slots[i])
                if i >= 1 and i - 1 < NR:
                    j = i - 1
                    pts[j] = ppt.tile([P, 2, 512], F16, tag="pt", bufs=6,
                                      name=f"pt{j}", uniquify=True)
                    nc.scalar.activation(pts[j], slots[j], EXP, scale=0.125)
                if i >= 2:
                    j = i - 2
                    rounds[j][1](pts[j])
                    del slots[j], pts[j]
                    for fn in posts.get(j, ()):
                        fn()

            # qt3 projection tail (no later rounds to hide it in)
            for st in range(4):
                proj_st(NQT - 1, st)

    nc.compile()
    return nc


def _get_nc():
    if "nc" not in _CACHE:
        _CACHE["nc"] = _build()
    return _CACHE["nc"]


def kernel(x, attention_mask, w_qkv, b_qkv, w_proj, b_proj, _trace=False):
    from concourse.bass_utils import run_bass_kernel_spmd

    x = np.asarray(x, dtype=np.float32)
    w_qkv = np.asarray(w_qkv, dtype=np.float32)
    b_qkv = np.asarray(b_qkv, dtype=np.float32)
    w_proj = np.asarray(w_proj, dtype=np.float32)
    b_proj = np.asarray(b_proj, dtype=np.float32)

    in_maps = []
    for core in range(NCORES):
        b, g = divmod(core, 4)
        base = g * 3 * DH
        wq2 = w_qkv[:, base + 2 * DH:base + 3 * DH]
        wk2 = w_qkv[:, D + base + 2 * DH:D + base + 3 * DH]
        bq2 = b_qkv[base + 2 * DH:base + 3 * DH]
        bk2 = b_qkv[D + base + 2 * DH:D + base + 3 * DH]
        f16 = np.float16
        in_maps.append({
            "xt": np.ascontiguousarray(x[b].T.astype(f16)),
            "wq01": np.ascontiguousarray(
                w_qkv[:, base:base + 2 * DH].astype(f16)),
            "wq2d": np.ascontiguousarray(
                np.concatenate([wq2, wq2], axis=1).astype(f16)),
            "wk01": np.ascontiguousarray(
                w_qkv[:, D + base:D + base + 2 * DH].astype(f16)),
            "wk2d": np.ascontiguousarray(
                np.concatenate([wk2, wk2], axis=1).astype(f16)),
            "wv": np.ascontiguousarray(
                w_qkv[:, 2 * D + base:2 * D + base + 3 * DH].astype(f16)),
            "bq01": np.ascontiguousarray(b_qkv[base:base + 2 * DH].reshape(P, 1)),
            "bq2d": np.ascontiguousarray(
                np.concatenate([bq2, bq2]).reshape(P, 1)),
            "bk01": np.ascontiguousarray(
                b_qkv[D + base:D + base + 2 * DH].reshape(P, 1)),
            "bk2d": np.ascontiguousarray(
                np.concatenate([bk2, bk2]).reshape(P, 1)),
            "bv": np.ascontiguousarray(
                b_qkv[2 * D + base:2 * D + base + 3 * DH].reshape(1, 3 * DH)),
            "wp0": np.ascontiguousarray(
                w_proj[base:base + DH, :].astype(f16)),
            "wp1": np.ascontiguousarray(
                w_proj[base + DH:base + 2 * DH, :].astype(f16)),
            "wp2": np.ascontiguousarray(
                w_proj[base + 2 * DH:base + 3 * DH, :].astype(f16)),
            "ones1": np.ones((1, 1), dtype=f16),
        })

    nc = _get_nc()
    # Warmup execution: the very first run after NEFF load can race the
    # ACT function-table load, corrupting a few exp results. Tables are
    # resident afterwards, so the second run is clean — return that one.
    run_bass_kernel_spmd(nc, in_maps, list(range(NCORES)), trace=False)
    res = run_bass_kernel_spmd(nc, in_maps, list(range(NCORES)), trace=_trace)
    if _trace:
        _CACHE["last_result"] = res

    out = np.zeros((B, S, D), dtype=np.float32)
    for core in range(NCORES):
        b = core // 4
        out[b] += res.results[core]["out"]
    out += b_proj[None, None, :]
    return out


# revision 44
# speedup vs baseline: 1.2433x; 1.2433x over previous
"""Multi-head attention (B=2, S=2048, D=768, H=12, Dh=64) on 8 TRN2 cores.

Sharding: core = (batch b = core//4, head-group g = core%4 of 3 heads).
Each core computes its 3 heads' attention for its batch and a partial
output projection [S, 768]; host sums the 4 group-partials per batch and
adds b_proj.

v2: fully software-pipelined single round structure.  All PE inputs are
f16 (half the DMA bytes and SBUF of f32).  Per (qt, key-chunk) "round":
two quadrant-paired score matmuls -> one exp on ACT -> two context
matmuls.  Rounds rotate through a 3-slot PSUM ring and a 6-slot SBUF pt
pool so no round has a dependency on the previous round's consumers
(the v1 kernel serialized on a pt-tile WAR and kept the PE at its
1.2 GHz mid P-state).  QKV streams, V, normalize and the projection are
interleaved into round "post" slots to keep both PE and ACT busy.
Normalize: in-place reciprocal_approx_fast on the Z row + PE ones-
broadcast into a ring slot + DVE multiply (v1 used a 3.3us DVE
reciprocal and a DRAM broadcast round-trip per head/qt).
"""

import numpy as np

B = 2
S = 2048
D = 768
NH = 12
DH = 64
NCORES = 8
P = 128
KCH = D // P          # 6 k-chunks for the QKV projection
NQT = S // 512        # 4 query tiles of 512
NKC = S // P          # 16 key chunks of 128

_CACHE = {}


def _build():
    import concourse.mybir as mybir
    import concourse.tile as tile
    from concourse import bacc

    from concourse.alu_op_type import AluOpType as ALU

    F32 = mybir.dt.float32
    F32R = mybir.dt.float32r
    F16 = mybir.dt.float16
    U32 = mybir.dt.uint32
    EXP = mybir.ActivationFunctionType.Exp

    nc = bacc.Bacc(target_bir_lowering=False, debug=False)

    xt_d = nc.dram_tensor("xt", [D, S], F16, kind="ExternalInput")
    wq01_d = nc.dram_tensor("wq01", [D, P], F16, kind="ExternalInput")
    wq2d_d = nc.dram_tensor("wq2d", [D, P], F16, kind="ExternalInput")
    wk01_d = nc.dram_tensor("wk01", [D, P], F16, kind="ExternalInput")
    wk2d_d = nc.dram_tensor("wk2d", [D, P], F16, kind="ExternalInput")
    wv_d = nc.dram_tensor("wv", [D, 3 * DH], F16, kind="ExternalInput")
    bq01_d = nc.dram_tensor("bq01", [P, 1], F32, kind="ExternalInput")
    bq2d_d = nc.dram_tensor("bq2d", [P, 1], F32, kind="ExternalInput")
    bk01_d = nc.dram_tensor("bk01", [P, 1], F32, kind="ExternalInput")
    bk2d_d = nc.dram_tensor("bk2d", [P, 1], F32, kind="ExternalInput")
    bv_d = nc.dram_tensor("bv", [1, 3 * DH], F32, kind="ExternalInput")
    wp_d = [nc.dram_tensor(f"wp{h}", [DH, D], F16, kind="ExternalInput")
            for h in range(3)]
    out_d = nc.dram_tensor("out", [S, D], F32, kind="ExternalOutput")

    with tile.TileContext(nc) as tc:
        with (
            tc.sbuf_pool(name="pw", bufs=1) as pw,
            tc.sbuf_pool(name="px", bufs=1) as px,
            tc.sbuf_pool(name="pqk", bufs=1) as pqk,
            tc.sbuf_pool(name="pv", bufs=1) as pv,
            tc.sbuf_pool(name="ppt", bufs=1) as ppt,
            tc.sbuf_pool(name="pctn", bufs=1) as pctn,
            tc.sbuf_pool(name="pz", bufs=1) as pz,
            tc.psum_pool(name="ps", bufs=1) as ps,
            tc.sbuf_pool(name="pout", bufs=3) as pout,
        ):
            # ---- weight / bias / input loads, spread across DMA queues ----
            wq01 = pw.tile([P, KCH, P], F16)
            wq2d = pw.tile([P, KCH, P], F16)
            wk01 = pw.tile([P, KCH, P], F16)
            wk2d = pw.tile([P, KCH, P], F16)
            wv = pw.tile([P, KCH, 3 * DH], F16)
            bq01 = pw.tile([P, 1], F32)
            bq2d = pw.tile([P, 1], F32)
            bk01 = pw.tile([P, 1], F32)
            bk2d = pw.tile([P, 1], F32)
            bvb = pw.tile([P, 3 * DH], F32)
            wp = [pw.tile([DH, D], F16, name=f"wp{h}") for h in range(3)]
            onesr = pw.tile([P, DH], F16)
            xt = px.tile([P, KCH, S], F16)
            xtr = xt_d.ap().rearrange("(c p) s -> c p s", p=P)

            # sync queue: k01 path first (first consumer), then xt c0/c1
            nc.sync.dma_start(out=bk01, in_=bk01_d.ap())
            nc.sync.dma_start(
                out=wk01, in_=wk01_d.ap().rearrange("(c p) m -> p c m", p=P))
            nc.sync.dma_start(out=xt[:, 0, :], in_=xtr[0])
            nc.sync.dma_start(out=xt[:, 1, :], in_=xtr[1])
            for h in range(3):
                nc.sync.dma_start(out=wp[h], in_=wp_d[h].ap())
            # gpsimd (Pool) queue
            nc.gpsimd.dma_start(out=bk2d, in_=bk2d_d.ap())
            nc.gpsimd.dma_start(
                out=wk2d, in_=wk2d_d.ap().rearrange("(c p) m -> p c m", p=P))
            nc.gpsimd.dma_start(out=xt[:, 2, :], in_=xtr[2])
            nc.gpsimd.dma_start(out=xt[:, 3, :], in_=xtr[3])
            nc.gpsimd.dma_start(out=bvb, in_=bv_d.ap().to_broadcast([P, 3 * DH]))
            # scalar (ACT) queue — startup only, ACT is otherwise idle here
            nc.scalar.dma_start(out=bq01, in_=bq01_d.ap())
            nc.scalar.dma_start(
                out=wq01, in_=wq01_d.ap().rearrange("(c p) m -> p c m", p=P))
            nc.scalar.dma_start(out=xt[:, 4, :], in_=xtr[4])
            nc.scalar.dma_start(out=xt[:, 5, :], in_=xtr[5])
            nc.scalar.dma_start(
                out=wq2d, in_=wq2d_d.ap().rearrange("(c p) m -> p c m", p=P))
            nc.scalar.dma_start(
                out=wv, in_=wv_d.ap().rearrange("(c p) m -> p c m", p=P))
            nc.scalar.dma_start(out=bq2d, in_=bq2d_d.ap())

            # ---- persistent SBUF tensors ----
            k01 = pqk.tile([P, S], F16)
            k2d = pqk.tile([P, S], F16)
            q01 = pqk.tile([P, S], F16)
            q2d = pqk.tile([P, S], F16)
            v3 = pv.tile([P, NKC, 3, DH + 1], F16)
            ctn = [pctn.tile([DH, NQT, 512], F16, name=f"ctn{h}")
                   for h in range(3)]
            warm = pz.tile([P, 1], F32, name="warm")
            warm16 = pz.tile([P, 1], F16, name="warm16")
            nc.vector.memset(onesr, 1.0)
            # v3's ones columns via memset: the v1/v2 DMA broadcast of a
            # [1,1] DRAM scalar into this scattered region took 20-40us
            # on hardware and stalled the first context matmuls.
            nc.gpsimd.memset(v3[:, :, :, DH:DH + 1], 1.0)

            def ring(name):
                return ps.tile([P, 2, 512], F32, tag="s", bufs=3, name=name,
                               uniquify=True)

            # accumulate in the order xt chunks land from the 3 DMA queues
            CORDER = [0, 2, 4, 1, 3, 5]

            def stream_chunk(dst, w, bias, qt, name):
                # one 512-wide slab of a QKV output stream
                acc = ring(name)
                for ci, c in enumerate(CORDER):
                    nc.tensor.matmul(
                        acc[:, 0, :], w[:, c, :],
                        xt[:, c, qt * 512:(qt + 1) * 512],
                        start=(ci == 0), stop=(ci == KCH - 1))
                nc.vector.tensor_scalar_add(
                    out=dst[:, qt * 512:(qt + 1) * 512], in0=acc[:, 0, :],
                    scalar1=bias)

            def v_group(sc):
                acc = ring(f"v{sc}")
                for c in range(KCH):
                    nc.tensor.matmul(
                        acc[:, 0, 0:3 * DH], xt[:, c, sc * P:(sc + 1) * P],
                        wv[:, c, :], start=(c == 0), stop=(c == KCH - 1))
                for h in range(3):
                    nc.vector.tensor_add(
                        v3[:, sc, h, 0:DH], acc[:, 0, h * DH:(h + 1) * DH],
                        bvb[:, h * DH:(h + 1) * DH])

            # ---- round definitions ----
            # one round = 2 paired score matmuls -> exp -> 2 ctx matmuls.
            # p01 rounds: heads 0,1 at key chunk c.  h2 rounds: head 2 at
            # key chunks (2rr, 2rr+1) via the duplicated k2d/q2d halves.
            ct = {}

            def make_p01(qt, c):
                def scores(slot):
                    q0 = qt * 512
                    nc.tensor.matmul(
                        slot[:, 0, :],
                        k01[0:DH, c * P:(c + 1) * P],
                        q01[0:DH, q0:q0 + 512], start=True, stop=True)
                    nc.tensor.matmul(
                        slot[:, 1, :],
                        k01[DH:P, c * P:(c + 1) * P],
                        q01[DH:P, q0:q0 + 512], start=True, stop=True)

                def ctx(pt):
                    if c == 0:
                        ct[(qt, 0)] = ps.tile([DH + 1, 512], F32, tag="ct",
                                              bufs=2, name=f"ct0_{qt}",
                                              uniquify=True)
                        ct[(qt, 1)] = ps.tile([DH + 1, 512], F32, tag="ct",
                                              bufs=2, name=f"ct1_{qt}",
                                              uniquify=True)
                    for h in range(2):
                        nc.tensor.matmul(
                            ct[(qt, h)], v3[:, c, h, :],
                            pt[:, h, :], start=(c == 0), stop=(c == NKC - 1))
                return scores, ctx

            def make_h2(qt, rr):
                c = 2 * rr

                def scores(slot):
                    q0 = qt * 512
                    nc.tensor.matmul(
                        slot[:, 0, :],
                        k2d[0:DH, c * P:(c + 1) * P],
                        q2d[0:DH, q0:q0 + 512], start=True, stop=True)
                    nc.tensor.matmul(
                        slot[:, 1, :],
                        k2d[DH:P, (c + 1) * P:(c + 2) * P],
                        q2d[DH:P, q0:q0 + 512], start=True, stop=True)

                def ctx(pt):
                    if rr == 0:
                        ct[(qt, 2)] = ps.tile([DH + 1, 512], F32, tag="ct",
                                              bufs=2, name=f"ct2_{qt}",
                                              uniquify=True)
                    for j in range(2):
                        nc.tensor.matmul(
                            ct[(qt, 2)], v3[:, c + j, 2, :],
                            pt[:, j, :], start=(rr == 0 and j == 0),
                            stop=(rr == NKC // 2 - 1 and j == 1))
                return scores, ctx

            pend = {}

            def norm_a(qt, heads):
                # DVE copies free the ct PSUM slots; in-place reciprocal
                # of the Z row, then an f16 copy of 1/Z for the f16
                # broadcast matmul in norm_b (spaced several rounds
                # later, so the PE never waits on this DVE chain).
                ctus = []
                for h in heads:
                    ctu = pz.tile([DH + 1, 512], F32R, tag="ctu", bufs=3,
                                  name=f"cu{h}{qt}", uniquify=True)
                    nc.vector.tensor_copy(ctu, ct[(qt, h)])
                    ctus.append(ctu)
                zs = []
                for j, h in enumerate(heads):
                    ctu = ctus[j]
                    zrow = ctu[DH:DH + 1, :].bitcast(F32)
                    nc.vector.reciprocal(out=zrow, in_=zrow)
                    zrec = pz.tile([DH + 1, 512], F16, tag="zr", bufs=3,
                                   name=f"zr{h}{qt}", uniquify=True)
                    nc.vector.tensor_copy(zrec[DH:DH + 1, :], zrow)
                    zs.append((h, ctu, zrec))
                pend[qt, heads[0]] = zs

            def norm_b(qt, h0):
                bc = ring(f"bc{qt}{h0}")
                for j, (h, ctu, zrec) in enumerate(pend.pop((qt, h0))):
                    nc.tensor.matmul(
                        bc[0:DH, j, :], onesr[DH:DH + 1, 0:DH],
                        zrec[DH:DH + 1, :], start=True, stop=True)
                    nc.vector.tensor_mul(
                        ctn[h][:, qt, :], ctu[0:DH, :].bitcast(F32),
                        bc[0:DH, j, :])

            def proj_st(qt, st):
                pp = ring(f"pp{qt}{st}")
                for r0, r1, reg in ((0, 512, 0), (512, D, 1)):
                    for h in range(3):
                        nc.tensor.matmul(
                            pp[:, reg, 0:r1 - r0],
                            ctn[h][:, qt, st * P:(st + 1) * P],
                            wp[h][:, r0:r1], start=(h == 0), stop=(h == 2))
                stage = pout.tile([P, D], F32, tag="stage",
                                  name=f"st{qt}{st}", uniquify=True)
                nc.vector.tensor_copy(stage[:, 0:512], pp[:, 0, :])
                nc.vector.tensor_copy(stage[:, 512:D], pp[:, 1, 0:256])
                r0 = qt * 512 + st * P
                q = nc.gpsimd if st % 2 == 0 else nc.sync
                q.dma_start(out=out_d.ap()[r0:r0 + P, :], in_=stage)

            # ---- build the global round list with post-work ----
            rounds = []
            posts = {}
            for qt in range(NQT):
                base = qt * 24
                for c in range(NKC):
                    rounds.append(make_p01(qt, c))
                for rr in range(NKC // 2):
                    rounds.append(make_h2(qt, rr))
                posts.setdefault(base + 15, []).append(
                    lambda qt=qt: norm_a(qt, [0, 1]))
                posts.setdefault(base + 21, []).append(
                    lambda qt=qt: norm_b(qt, 0))
                posts.setdefault(base + 23, []).append(
                    lambda qt=qt: norm_a(qt, [2]))
                if qt < NQT - 1:
                    posts.setdefault(base + 27, []).append(
                        lambda qt=qt: norm_b(qt, 2))
                if qt < NQT - 1:
                    # spread proj st-chunks well into the next qt's rounds
                    # so they never wait on the normalize chain from the
                    # in-order PE stream
                    for st in range(4):
                        posts.setdefault(base + 33 + 4 * st, []).append(
                            lambda qt=qt, st=st: proj_st(qt, st))
            # qt0 fill-in: V groups and remaining QKV streams
            posts[0] = [lambda: v_group(2), lambda: v_group(3)]
            posts[1] = [lambda: stream_chunk(q2d, wq2d, bq2d, 0, "q2d0")]
            posts[2] = [lambda: v_group(4), lambda: v_group(5)]
            posts[3] = [lambda: stream_chunk(k2d, wk2d, bk2d, 0, "k2d0")]
            posts[4] = [lambda: v_group(6), lambda: v_group(7)]
            posts[5] = [lambda: stream_chunk(k2d, wk2d, bk2d, 1, "k2d1")]
            posts[6] = [lambda: v_group(8), lambda: v_group(9)]
            posts[7] = [lambda: stream_chunk(k2d, wk2d, bk2d, 2, "k2d2")]
            posts[8] = [lambda: v_group(10), lambda: v_group(11)]
            posts[9] = [lambda: stream_chunk(k2d, wk2d, bk2d, 3, "k2d3")]
            posts[10] = [lambda: v_group(12), lambda: v_group(13)]
            posts[11] = [lambda: stream_chunk(q01, wq01, bq01, 1, "q011")]
            posts[12] = [lambda: v_group(14), lambda: v_group(15)]
            posts[13] = [lambda: stream_chunk(q2d, wq2d, bq2d, 1, "q2d1")]
            # later qt fill-in: next qt's q streams
            for qt in (1, 2):
                posts.setdefault(qt * 24 + 3, []).append(
                    lambda qt=qt: stream_chunk(q01, wq01, bq01, qt + 1,
                                               f"q01{qt + 1}"))
                posts.setdefault(qt * 24 + 9, []).append(
                    lambda qt=qt: stream_chunk(q2d, wq2d, bq2d, qt + 1,
                                               f"q2d{qt + 1}"))

            # k01 slabs 1-3 land mid-stream, just before the first scores
            # round that reads them (round 4s reads slab s)
            pres = {2: [lambda: stream_chunk(k01, wk01, bk01, 1, "k011")],
                    6: [lambda: stream_chunk(k01, wk01, bk01, 2, "k012")],
                    10: [lambda: stream_chunk(k01, wk01, bk01, 3, "k013")]}

            # ---- prologue: only what rounds 0-3 need ----
            stream_chunk(k01, wk01, bk01, 0, "k010")
            stream_chunk(q01, wq01, bq01, 0, "q010")
            v_group(0)
            v_group(1)
            # warm the ACT exp table so round 0 doesn't pay the load
            nc.vector.memset(warm, 1.0)
            nc.scalar.activation(warm16, warm, EXP, scale=0.125)

            # ---- software-pipelined main loop ----
            NR = len(rounds)
            slots = {}
            pts = {}
            for i in range(NR + 2):
                for fn in pres.get(i, ()):
                    fn()
                if i < NR:
                    slots[i] = ring(f"r{i}")
                    rounds[i][0](slots[i])
                if i >= 1 and i - 1 < NR:
                    j = i - 1
                    pts[j] = ppt.tile([P, 2, 512], F16, tag="pt", bufs=6,
                                      name=f"pt{j}", uniquify=True)
                    nc.scalar.activation(pts[j], slots[j], EXP, scale=0.125)
                if i >= 2:
                    j = i - 2
                    rounds[j][1](pts[j])
                    del slots[j], pts[j]
                    for fn in posts.get(j, ()):
                        fn()

            # qt3 normalize-b + projection tail (no later rounds)
            norm_b(NQT - 1, 2)
            for st in range(4):
                proj_st(NQT - 1, st)

    nc.compile()
    return nc


def _get_nc():
    if "nc" not in _CACHE:
        _CACHE["nc"] = _build()
    return _CACHE["nc"]


def kernel(x, attention_mask, w_qkv, b_qkv, w_proj, b_proj, _trace=False):
    from concourse.bass_utils import run_bass_kernel_spmd

    x = np.asarray(x, dtype=np.float32)
    w_qkv = np.asarray(w_qkv, dtype=np.float32)
    b_qkv = np.asarray(b_qkv, dtype=np.float32)
    w_proj = np.asarray(w_proj, dtype=np.float32)
    b_proj = np.asarray(b_proj, dtype=np.float32)

    in_maps = []
    for core in range(NCORES):
        b, g = divmod(core, 4)
        base = g * 3 * DH
        wq2 = w_qkv[:, base + 2 * DH:base + 3 * DH]
        wk2 = w_qkv[:, D + base + 2 * DH:D + base + 3 * DH]
        bq2 = b_qkv[base + 2 * DH:base + 3 * DH]
        bk2 = b_qkv[D + base + 2 * DH:D + base + 3 * DH]
        f16 = np.float16
        in_maps.append({
            "xt": np.ascontiguousarray(x[b].T.astype(f16)),
            "wq01": np.ascontiguousarray(
                w_qkv[:, base:base + 2 * DH].astype(f16)),
            "wq2d": np.ascontiguousarray(
                np.concatenate([wq2, wq2], axis=1).astype(f16)),
            "wk01": np.ascontiguousarray(
                w_qkv[:, D + base:D + base + 2 * DH].astype(f16)),
            "wk2d": np.ascontiguousarray(
                np.concatenate([wk2, wk2], axis=1).astype(f16)),
            "wv": np.ascontiguousarray(
                w_qkv[:, 2 * D + base:2 * D + base + 3 * DH].astype(f16)),
            "bq01": np.ascontiguousarray(b_qkv[base:base + 2 * DH].reshape(P, 1)),
            "bq2d": np.ascontiguousarray(
                np.concatenate([bq2, bq2]).reshape(P, 1)),
            "bk01": np.ascontiguousarray(
                b_qkv[D + base:D + base + 2 * DH].reshape(P, 1)),
            "bk2d": np.ascontiguousarray(
                np.concatenate([bk2, bk2]).reshape(P, 1)),
            "bv": np.ascontiguousarray(
                b_qkv[2 * D + base:2 * D + base + 3 * DH].reshape(1, 3 * DH)),
            "wp0": np.ascontiguousarray(
                w_proj[base:base + DH, :].astype(f16)),
            "wp1": np.ascontiguousarray(
                w_proj[base + DH:base + 2 * DH, :].astype(f16)),
            "wp2": np.ascontiguousarray(
                w_proj[base + 2 * DH:base + 3 * DH, :].astype(f16)),
        })

    nc = _get_nc()
    # Warmup execution: the very first run after NEFF load can race the
    # ACT function-table load, corrupting a few exp results. Tables are
    # resident afterwards, so the second run is clean — return that one.
    run_bass_kernel_spmd(nc, in_maps, list(range(NCORES)), trace=False)
    res = run_bass_kernel_spmd(nc, in_maps, list(range(NCORES)), trace=_trace)
    if _trace:
        _CACHE["last_result"] = res

    out = np.zeros((B, S, D), dtype=np.float32)
    for core in range(NCORES):
        b = core // 4
        out[b] += res.results[core]["out"]
    out += b_proj[None, None, :]
    return out


# revision 46
# speedup vs baseline: 1.3788x; 1.1090x over previous
"""Multi-head attention (B=2, S=2048, D=768, H=12, Dh=64) on 8 TRN2 cores.

Sharding: core = (batch b = core//4, head-group g = core%4 of 3 heads).
Each core computes its 3 heads' attention for its batch and a partial
output projection [S, 768]; host sums the 4 group-partials per batch and
adds b_proj.

v2: fully software-pipelined single round structure.  All PE inputs are
f16 (half the DMA bytes and SBUF of f32).  Per (qt, key-chunk) "round":
two quadrant-paired score matmuls -> one exp on ACT -> two context
matmuls.  Rounds rotate through a 3-slot PSUM ring and a 6-slot SBUF pt
pool so no round has a dependency on the previous round's consumers
(the v1 kernel serialized on a pt-tile WAR and kept the PE at its
1.2 GHz mid P-state).  QKV streams, V, normalize and the projection are
interleaved into round "post" slots to keep both PE and ACT busy.
Normalize: in-place reciprocal_approx_fast on the Z row + PE ones-
broadcast into a ring slot + DVE multiply (v1 used a 3.3us DVE
reciprocal and a DRAM broadcast round-trip per head/qt).
"""

import numpy as np

B = 2
S = 2048
D = 768
NH = 12
DH = 64
NCORES = 8
P = 128
KCH = D // P          # 6 k-chunks for the QKV projection
NQT = S // 512        # 4 query tiles of 512
NKC = S // P          # 16 key chunks of 128

_CACHE = {}


def _build():
    import concourse.mybir as mybir
    import concourse.tile as tile
    from concourse import bacc

    from concourse.alu_op_type import AluOpType as ALU

    F32 = mybir.dt.float32
    F32R = mybir.dt.float32r
    F16 = mybir.dt.float16
    U32 = mybir.dt.uint32
    EXP = mybir.ActivationFunctionType.Exp

    nc = bacc.Bacc(target_bir_lowering=False, debug=False)

    xt_d = nc.dram_tensor("xt", [D, S], F16, kind="ExternalInput")
    wq01_d = nc.dram_tensor("wq01", [D, P], F16, kind="ExternalInput")
    wq2d_d = nc.dram_tensor("wq2d", [D, P], F16, kind="ExternalInput")
    wk01_d = nc.dram_tensor("wk01", [D, P], F16, kind="ExternalInput")
    wk2d_d = nc.dram_tensor("wk2d", [D, P], F16, kind="ExternalInput")
    wv_d = nc.dram_tensor("wv", [D, 3 * DH], F16, kind="ExternalInput")
    bq01_d = nc.dram_tensor("bq01", [P, 1], F32, kind="ExternalInput")
    bq2d_d = nc.dram_tensor("bq2d", [P, 1], F32, kind="ExternalInput")
    bk01_d = nc.dram_tensor("bk01", [P, 1], F32, kind="ExternalInput")
    bk2d_d = nc.dram_tensor("bk2d", [P, 1], F32, kind="ExternalInput")
    bv_d = nc.dram_tensor("bv", [1, 3 * DH], F32, kind="ExternalInput")
    wp_d = [nc.dram_tensor(f"wp{h}", [DH, D], F16, kind="ExternalInput")
            for h in range(3)]
    out_d = nc.dram_tensor("out", [S, D], F32, kind="ExternalOutput")

    with tile.TileContext(nc) as tc:
        with (
            tc.sbuf_pool(name="pw", bufs=1) as pw,
            tc.sbuf_pool(name="px", bufs=1) as px,
            tc.sbuf_pool(name="pqk", bufs=1) as pqk,
            tc.sbuf_pool(name="pv", bufs=1) as pv,
            tc.sbuf_pool(name="ppt", bufs=1) as ppt,
            tc.sbuf_pool(name="pctn", bufs=1) as pctn,
            tc.sbuf_pool(name="pz", bufs=1) as pz,
            tc.psum_pool(name="ps", bufs=1) as ps,
            tc.tile_pool(name="pdram", bufs=2, space="DRAM") as pdram,
            tc.sbuf_pool(name="pout", bufs=3) as pout,
        ):
            # ---- weight / bias / input loads, spread across DMA queues ----
            wq01 = pw.tile([P, KCH, P], F16)
            wq2d = pw.tile([P, KCH, P], F16)
            wk01 = pw.tile([P, KCH, P], F16)
            wk2d = pw.tile([P, KCH, P], F16)
            wv = pw.tile([P, KCH, 3 * DH], F16)
            bq01 = pw.tile([P, 1], F32)
            bq2d = pw.tile([P, 1], F32)
            bk01 = pw.tile([P, 1], F32)
            bk2d = pw.tile([P, 1], F32)
            bvb = pw.tile([P, 3 * DH], F32)
            wp = [pw.tile([DH, D], F16, name=f"wp{h}") for h in range(3)]
            xt = px.tile([P, KCH, S], F16)
            xtr = xt_d.ap().rearrange("(c p) s -> c p s", p=P)

            # sync queue: k01 path first (first consumer), then xt c0/c1
            nc.sync.dma_start(out=bk01, in_=bk01_d.ap())
            nc.sync.dma_start(
                out=wk01, in_=wk01_d.ap().rearrange("(c p) m -> p c m", p=P))
            nc.sync.dma_start(out=xt[:, 0, :], in_=xtr[0])
            nc.sync.dma_start(out=xt[:, 1, :], in_=xtr[1])
            for h in range(3):
                nc.sync.dma_start(out=wp[h], in_=wp_d[h].ap())
            # gpsimd (Pool) queue
            nc.gpsimd.dma_start(out=bk2d, in_=bk2d_d.ap())
            nc.gpsimd.dma_start(
                out=wk2d, in_=wk2d_d.ap().rearrange("(c p) m -> p c m", p=P))
            nc.gpsimd.dma_start(out=xt[:, 2, :], in_=xtr[2])
            nc.gpsimd.dma_start(out=xt[:, 3, :], in_=xtr[3])
            nc.gpsimd.dma_start(out=bvb, in_=bv_d.ap().to_broadcast([P, 3 * DH]))
            # scalar (ACT) queue — startup only, ACT is otherwise idle here
            nc.scalar.dma_start(out=bq01, in_=bq01_d.ap())
            nc.scalar.dma_start(
                out=wq01, in_=wq01_d.ap().rearrange("(c p) m -> p c m", p=P))
            nc.scalar.dma_start(out=xt[:, 4, :], in_=xtr[4])
            nc.scalar.dma_start(out=xt[:, 5, :], in_=xtr[5])
            nc.scalar.dma_start(
                out=wq2d, in_=wq2d_d.ap().rearrange("(c p) m -> p c m", p=P))
            nc.scalar.dma_start(
                out=wv, in_=wv_d.ap().rearrange("(c p) m -> p c m", p=P))
            nc.scalar.dma_start(out=bq2d, in_=bq2d_d.ap())

            # ---- persistent SBUF tensors ----
            k01 = pqk.tile([P, S], F16)
            k2d = pqk.tile([P, S], F16)
            q01 = pqk.tile([P, S], F16)
            q2d = pqk.tile([P, S], F16)
            v3 = pv.tile([P, NKC, 3, DH + 1], F16)
            ctn = [pctn.tile([DH, NQT, 512], F16, name=f"ctn{h}")
                   for h in range(3)]
            warm = pz.tile([P, 1], F32, name="warm")
            warm16 = pz.tile([P, 1], F16, name="warm16")
            # v3's ones columns via memset: the v1/v2 DMA broadcast of a
            # [1,1] DRAM scalar into this scattered region took 20-40us
            # on hardware and stalled the first context matmuls.
            nc.gpsimd.memset(v3[:, :, :, DH:DH + 1], 1.0)

            def ring(name):
                return ps.tile([P, 2, 512], F32, tag="s", bufs=3, name=name,
                               uniquify=True)

            # accumulate in the order xt chunks land from the 3 DMA queues
            CORDER = [0, 2, 4, 1, 3, 5]

            def stream_chunk(dst, w, bias, qt, name):
                # one 512-wide slab of a QKV output stream
                acc = ring(name)
                for ci, c in enumerate(CORDER):
                    nc.tensor.matmul(
                        acc[:, 0, :], w[:, c, :],
                        xt[:, c, qt * 512:(qt + 1) * 512],
                        start=(ci == 0), stop=(ci == KCH - 1))
                nc.vector.tensor_scalar_add(
                    out=dst[:, qt * 512:(qt + 1) * 512], in0=acc[:, 0, :],
                    scalar1=bias)

            def v_group(sc):
                acc = ring(f"v{sc}")
                for c in range(KCH):
                    nc.tensor.matmul(
                        acc[:, 0, 0:3 * DH], xt[:, c, sc * P:(sc + 1) * P],
                        wv[:, c, :], start=(c == 0), stop=(c == KCH - 1))
                for h in range(3):
                    nc.vector.tensor_add(
                        v3[:, sc, h, 0:DH], acc[:, 0, h * DH:(h + 1) * DH],
                        bvb[:, h * DH:(h + 1) * DH])

            # ---- round definitions ----
            # one round = 2 paired score matmuls -> exp -> 2 ctx matmuls.
            # p01 rounds: heads 0,1 at key chunk c.  h2 rounds: head 2 at
            # key chunks (2rr, 2rr+1) via the duplicated k2d/q2d halves.
            ct = {}

            def make_p01(qt, c):
                def scores(slot):
                    q0 = qt * 512
                    nc.tensor.matmul(
                        slot[:, 0, :],
                        k01[0:DH, c * P:(c + 1) * P],
                        q01[0:DH, q0:q0 + 512], start=True, stop=True)
                    nc.tensor.matmul(
                        slot[:, 1, :],
                        k01[DH:P, c * P:(c + 1) * P],
                        q01[DH:P, q0:q0 + 512], start=True, stop=True)

                def ctx(pt):
                    if c == 0:
                        ct[(qt, 0)] = ps.tile([DH + 1, 512], F32, tag="ct",
                                              bufs=2, name=f"ct0_{qt}",
                                              uniquify=True)
                        ct[(qt, 1)] = ps.tile([DH + 1, 512], F32, tag="ct",
                                              bufs=2, name=f"ct1_{qt}",
                                              uniquify=True)
                    for h in range(2):
                        nc.tensor.matmul(
                            ct[(qt, h)], v3[:, c, h, :],
                            pt[:, h, :], start=(c == 0), stop=(c == NKC - 1))
                return scores, ctx

            def make_h2(qt, rr):
                c = 2 * rr

                def scores(slot):
                    q0 = qt * 512
                    nc.tensor.matmul(
                        slot[:, 0, :],
                        k2d[0:DH, c * P:(c + 1) * P],
                        q2d[0:DH, q0:q0 + 512], start=True, stop=True)
                    nc.tensor.matmul(
                        slot[:, 1, :],
                        k2d[DH:P, (c + 1) * P:(c + 2) * P],
                        q2d[DH:P, q0:q0 + 512], start=True, stop=True)

                def ctx(pt):
                    if rr == 0:
                        ct[(qt, 2)] = ps.tile([DH + 1, 512], F32, tag="ct",
                                              bufs=2, name=f"ct2_{qt}",
                                              uniquify=True)
                    for j in range(2):
                        nc.tensor.matmul(
                            ct[(qt, 2)], v3[:, c + j, 2, :],
                            pt[:, j, :], start=(rr == 0 and j == 0),
                            stop=(rr == NKC // 2 - 1 and j == 1))
                return scores, ctx

            pend = {}

            def norm_a(qt, heads, direct=False):
                # DVE copies free the ct PSUM slots.  1/Z: the Z rows are
                # bounced to DRAM and re-read spread over 128 partitions,
                # so the reciprocal costs ~4 elements/partition instead
                # of a 3.3us single-partition crawl that head-of-line
                # blocks the in-order DVE queue.  Broadcast back via the
                # DRAM to_broadcast read.  No PE or PSUM involvement.
                ctus = []
                for h in heads:
                    ctu = pz.tile([DH + 1, 512], F32R, tag="ctu", bufs=3,
                                  name=f"cu{h}{qt}", uniquify=True)
                    nc.vector.tensor_copy(ctu, ct[(qt, h)])
                    ctus.append(ctu)
                n = len(heads)
                zs = []
                if direct:
                    # kernel tail: latency beats throughput, DVE is idle
                    for j, h in enumerate(heads):
                        zrow = ctus[j][DH:DH + 1, :].bitcast(F32)
                        nc.vector.reciprocal(out=zrow, in_=zrow)
                        zd = pdram.tile([1, 512], F32, tag="zd",
                                        name=f"zd{h}{qt}", uniquify=True)
                        nc.sync.dma_start(out=zd, in_=zrow)
                        rp = pz.tile([DH, 512], F32, tag="rp", bufs=3,
                                     name=f"rp{h}{qt}", uniquify=True)
                        nc.gpsimd.dma_start(
                            out=rp, in_=zd.to_broadcast([DH, 512]))
                        zs.append((h, ctus[j], rp))
                else:
                    zg = pz.tile([P, 4 * n], F32, tag="zg", bufs=2,
                                 name=f"zg{qt}{heads[0]}", uniquify=True)
                    zd = pdram.tile([P, 4 * n], F32, tag="zd",
                                    name=f"zd{qt}{heads[0]}", uniquify=True)
                    zflat = zd.rearrange("p f -> (p f)")
                    for j, h in enumerate(heads):
                        nc.sync.dma_start(
                            out=zflat[j * 512:(j + 1) * 512].rearrange(
                                "(o q) -> o q", o=1),
                            in_=ctus[j][DH:DH + 1, :].bitcast(F32))
                    nc.sync.dma_start(out=zg, in_=zd)
                    nc.vector.reciprocal(out=zg, in_=zg)
                    zd2 = pdram.tile([P, 4 * n], F32, tag="zd2",
                                     name=f"ze{qt}{heads[0]}", uniquify=True)
                    nc.sync.dma_start(out=zd2, in_=zg)
                    zflat2 = zd2.rearrange("p f -> (p f)")
                    for j, h in enumerate(heads):
                        rp = pz.tile([DH, 512], F32, tag="rp", bufs=3,
                                     name=f"rp{h}{qt}", uniquify=True)
                        nc.gpsimd.dma_start(
                            out=rp,
                            in_=zflat2[j * 512:(j + 1) * 512].rearrange(
                                "(o q) -> o q", o=1).to_broadcast([DH, 512]))
                        zs.append((h, ctus[j], rp))
                pend[qt, heads[0]] = zs

            def norm_b(qt, h0):
                for h, ctu, rp in pend.pop((qt, h0)):
                    nc.vector.tensor_mul(
                        ctn[h][:, qt, :], ctu[0:DH, :].bitcast(F32), rp)

            def proj_st(qt, st):
                pp = ring(f"pp{qt}{st}")
                for r0, r1, reg in ((0, 512, 0), (512, D, 1)):
                    for h in range(3):
                        nc.tensor.matmul(
                            pp[:, reg, 0:r1 - r0],
                            ctn[h][:, qt, st * P:(st + 1) * P],
                            wp[h][:, r0:r1], start=(h == 0), stop=(h == 2))
                stage = pout.tile([P, D], F32, tag="stage",
                                  name=f"st{qt}{st}", uniquify=True)
                nc.vector.tensor_copy(stage[:, 0:512], pp[:, 0, :])
                nc.vector.tensor_copy(stage[:, 512:D], pp[:, 1, 0:256])
                r0 = qt * 512 + st * P
                q = nc.gpsimd if st % 2 == 0 else nc.sync
                q.dma_start(out=out_d.ap()[r0:r0 + P, :], in_=stage)

            # ---- build the global round list with post-work ----
            rounds = []
            posts = {}
            for qt in range(NQT):
                base = qt * 24
                for c in range(NKC):
                    rounds.append(make_p01(qt, c))
                for rr in range(NKC // 2):
                    rounds.append(make_h2(qt, rr))
                posts.setdefault(base + 15, []).append(
                    lambda qt=qt: norm_a(qt, [0, 1]))
                posts.setdefault(base + 21, []).append(
                    lambda qt=qt: norm_b(qt, 0))
                posts.setdefault(base + 23, []).append(
                    lambda qt=qt: norm_a(qt, [2], direct=(qt == NQT - 1)))
                if qt < NQT - 1:
                    posts.setdefault(base + 29, []).append(
                        lambda qt=qt: norm_b(qt, 2))
                if qt < NQT - 1:
                    # spread proj st-chunks well into the next qt's rounds
                    # so they never wait on the normalize chain from the
                    # in-order PE stream
                    for st in range(4):
                        posts.setdefault(base + 33 + 4 * st, []).append(
                            lambda qt=qt, st=st: proj_st(qt, st))
            # qt0 fill-in: V groups and remaining QKV streams
            posts[0] = [lambda: v_group(2), lambda: v_group(3)]
            posts[1] = [lambda: stream_chunk(q2d, wq2d, bq2d, 0, "q2d0")]
            posts[2] = [lambda: v_group(4), lambda: v_group(5)]
            posts[3] = [lambda: stream_chunk(k2d, wk2d, bk2d, 0, "k2d0")]
            posts[4] = [lambda: v_group(6), lambda: v_group(7)]
            posts[5] = [lambda: stream_chunk(k2d, wk2d, bk2d, 1, "k2d1")]
            posts[6] = [lambda: v_group(8), lambda: v_group(9)]
            posts[7] = [lambda: stream_chunk(k2d, wk2d, bk2d, 2, "k2d2")]
            posts[8] = [lambda: v_group(10), lambda: v_group(11)]
            posts[9] = [lambda: stream_chunk(k2d, wk2d, bk2d, 3, "k2d3")]
            posts[10] = [lambda: v_group(12), lambda: v_group(13)]
            posts[11] = [lambda: stream_chunk(q01, wq01, bq01, 1, "q011")]
            posts[12] = [lambda: v_group(14), lambda: v_group(15)]
            posts[13] = [lambda: stream_chunk(q2d, wq2d, bq2d, 1, "q2d1")]
            # later qt fill-in: next qt's q streams
            for qt in (1, 2):
                posts.setdefault(qt * 24 + 3, []).append(
                    lambda qt=qt: stream_chunk(q01, wq01, bq01, qt + 1,
                                               f"q01{qt + 1}"))
                posts.setdefault(qt * 24 + 9, []).append(
                    lambda qt=qt: stream_chunk(q2d, wq2d, bq2d, qt + 1,
                                               f"q2d{qt + 1}"))

            # k01 slabs 1-3 land mid-stream, just before the first scores
            # round that reads them (round 4s reads slab s)
            pres = {2: [lambda: stream_chunk(k01, wk01, bk01, 1, "k011")],
                    6: [lambda: stream_chunk(k01, wk01, bk01, 2, "k012")],
                    10: [lambda: stream_chunk(k01, wk01, bk01, 3, "k013")]}

            # ---- prologue: only what rounds 0-3 need ----
            stream_chunk(k01, wk01, bk01, 0, "k010")
            stream_chunk(q01, wq01, bq01, 0, "q010")
            v_group(0)
            v_group(1)
            # warm the ACT exp table so round 0 doesn't pay the load
            nc.vector.memset(warm, 1.0)
            nc.scalar.activation(warm16, warm, EXP, scale=0.125)

            # ---- software-pipelined main loop ----
            NR = len(rounds)
            slots = {}
            pts = {}
            for i in range(NR + 2):
                for fn in pres.get(i, ()):
                    fn()
                if i < NR:
                    slots[i] = ring(f"r{i}")
                    rounds[i][0](slots[i])
                if i >= 1 and i - 1 < NR:
                    j = i - 1
                    pts[j] = ppt.tile([P, 2, 512], F16, tag="pt", bufs=6,
                                      name=f"pt{j}", uniquify=True)
                    nc.scalar.activation(pts[j], slots[j], EXP, scale=0.125)
                if i >= 2:
                    j = i - 2
                    rounds[j][1](pts[j])
                    del slots[j], pts[j]
                    for fn in posts.get(j, ()):
                        fn()

            # qt3 normalize-b + projection tail (no later rounds)
            norm_b(NQT - 1, 2)
            for st in range(4):
                proj_st(NQT - 1, st)

    nc.compile()
    return nc


def _get_nc():
    if "nc" not in _CACHE:
        _CACHE["nc"] = _build()
    return _CACHE["nc"]


def kernel(x, attention_mask, w_qkv, b_qkv, w_proj, b_proj, _trace=False):
    from concourse.bass_utils import run_bass_kernel_spmd

    x = np.asarray(x, dtype=np.float32)
    w_qkv = np.asarray(w_qkv, dtype=np.float32)
    b_qkv = np.asarray(b_qkv, dtype=np.float32)
    w_proj = np.asarray(w_proj, dtype=np.float32)
    b_proj = np.asarray(b_proj, dtype=np.float32)

    in_maps = []
    for core in range(NCORES):
        b, g = divmod(core, 4)
        base = g * 3 * DH
        wq2 = w_qkv[:, base + 2 * DH:base + 3 * DH]
        wk2 = w_qkv[:, D + base + 2 * DH:D + base + 3 * DH]
        bq2 = b_qkv[base + 2 * DH:base + 3 * DH]
        bk2 = b_qkv[D + base + 2 * DH:D + base + 3 * DH]
        f16 = np.float16
        in_maps.append({
            "xt": np.ascontiguousarray(x[b].T.astype(f16)),
            "wq01": np.ascontiguousarray(
                w_qkv[:, base:base + 2 * DH].astype(f16)),
            "wq2d": np.ascontiguousarray(
                np.concatenate([wq2, wq2], axis=1).astype(f16)),
            "wk01": np.ascontiguousarray(
                w_qkv[:, D + base:D + base + 2 * DH].astype(f16)),
            "wk2d": np.ascontiguousarray(
                np.concatenate([wk2, wk2], axis=1).astype(f16)),
            "wv": np.ascontiguousarray(
                w_qkv[:, 2 * D + base:2 * D + base + 3 * DH].astype(f16)),
            "bq01": np.ascontiguousarray(b_qkv[base:base + 2 * DH].reshape(P, 1)),
            "bq2d": np.ascontiguousarray(
                np.concatenate([bq2, bq2]).reshape(P, 1)),
            "bk01": np.ascontiguousarray(
                b_qkv[D + base:D + base + 2 * DH].reshape(P, 1)),
            "bk2d": np.ascontiguousarray(
                np.concatenate([bk2, bk2]).reshape(P, 1)),
            "bv": np.ascontiguousarray(
                b_qkv[2 * D + base:2 * D + base + 3 * DH].reshape(1, 3 * DH)),
            "wp0": np.ascontiguousarray(
                w_proj[base:base + DH, :].astype(f16)),
            "wp1": np.ascontiguousarray(
                w_proj[base + DH:base + 2 * DH, :].astype(f16)),
            "wp2": np.ascontiguousarray(
                w_proj[base + 2 * DH:base + 3 * DH, :].astype(f16)),
        })

    nc = _get_nc()
    # Warmup execution: the very first run after NEFF load can race the
    # ACT function-table load, corrupting a few exp results. Tables are
    # resident afterwards, so the second run is clean — return that one.
    run_bass_kernel_spmd(nc, in_maps, list(range(NCORES)), trace=False)
    res = run_bass_kernel_spmd(nc, in_maps, list(range(NCORES)), trace=_trace)
    if _trace:
        _CACHE["last_result"] = res

    out = np.zeros((B, S, D), dtype=np.float32)
    for core in range(NCORES):
        b = core // 4
        out[b] += res.results[core]["out"]
    out += b_proj[None, None, :]
    return out


# revision 48
# speedup vs baseline: 1.4069x; 1.0204x over previous
"""Multi-head attention (B=2, S=2048, D=768, H=12, Dh=64) on 8 TRN2 cores.

Sharding: core = (batch b = core//4, head-group g = core%4 of 3 heads).
Each core computes its 3 heads' attention for its batch and a partial
output projection [S, 768]; host sums the 4 group-partials per batch and
adds b_proj.

v2: fully software-pipelined single round structure.  All PE inputs are
f16 (half the DMA bytes and SBUF of f32).  Per (qt, key-chunk) "round":
two quadrant-paired score matmuls -> one exp on ACT -> two context
matmuls.  Rounds rotate through a 3-slot PSUM ring and a 6-slot SBUF pt
pool so no round has a dependency on the previous round's consumers
(the v1 kernel serialized on a pt-tile WAR and kept the PE at its
1.2 GHz mid P-state).  QKV streams, V, normalize and the projection are
interleaved into round "post" slots to keep both PE and ACT busy.
Normalize: in-place reciprocal_approx_fast on the Z row + PE ones-
broadcast into a ring slot + DVE multiply (v1 used a 3.3us DVE
reciprocal and a DRAM broadcast round-trip per head/qt).
"""

import numpy as np

B = 2
S = 2048
D = 768
NH = 12
DH = 64
NCORES = 8
P = 128
KCH = D // P          # 6 k-chunks for the QKV projection
NQT = S // 512        # 4 query tiles of 512
NKC = S // P          # 16 key chunks of 128

_CACHE = {}


def _build():
    import concourse.mybir as mybir
    import concourse.tile as tile
    from concourse import bacc

    from concourse.alu_op_type import AluOpType as ALU

    F32 = mybir.dt.float32
    F32R = mybir.dt.float32r
    F16 = mybir.dt.float16
    U32 = mybir.dt.uint32
    EXP = mybir.ActivationFunctionType.Exp

    nc = bacc.Bacc(target_bir_lowering=False, debug=False)

    xt_d = nc.dram_tensor("xt", [D, S], F16, kind="ExternalInput")
    wq01_d = nc.dram_tensor("wq01", [D, P], F16, kind="ExternalInput")
    wq2d_d = nc.dram_tensor("wq2d", [D, P], F16, kind="ExternalInput")
    wk01_d = nc.dram_tensor("wk01", [D, P], F16, kind="ExternalInput")
    wk2d_d = nc.dram_tensor("wk2d", [D, P], F16, kind="ExternalInput")
    wv_d = nc.dram_tensor("wv", [D, 3 * DH], F16, kind="ExternalInput")
    bq01_d = nc.dram_tensor("bq01", [P, 1], F32, kind="ExternalInput")
    bq2d_d = nc.dram_tensor("bq2d", [P, 1], F32, kind="ExternalInput")
    bk01_d = nc.dram_tensor("bk01", [P, 1], F32, kind="ExternalInput")
    bk2d_d = nc.dram_tensor("bk2d", [P, 1], F32, kind="ExternalInput")
    bv_d = nc.dram_tensor("bv", [1, 3 * DH], F32, kind="ExternalInput")
    wp01_d = nc.dram_tensor("wp01", [P, D], F16, kind="ExternalInput")
    wp2_d = nc.dram_tensor("wp2", [DH, D], F16, kind="ExternalInput")
    out_d = nc.dram_tensor("out", [S, D], F32, kind="ExternalOutput")

    with tile.TileContext(nc) as tc:
        with (
            tc.sbuf_pool(name="pw", bufs=1) as pw,
            tc.sbuf_pool(name="px", bufs=1) as px,
            tc.sbuf_pool(name="pqk", bufs=1) as pqk,
            tc.sbuf_pool(name="pv", bufs=1) as pv,
            tc.sbuf_pool(name="ppt", bufs=1) as ppt,
            tc.sbuf_pool(name="pctn", bufs=1) as pctn,
            tc.sbuf_pool(name="pz", bufs=1) as pz,
            tc.psum_pool(name="ps", bufs=1) as ps,
            tc.tile_pool(name="pdram", bufs=2, space="DRAM") as pdram,
            tc.sbuf_pool(name="pout", bufs=3) as pout,
        ):
            # ---- weight / bias / input loads, spread across DMA queues ----
            wq01 = pw.tile([P, KCH, P], F16)
            wq2d = pw.tile([P, KCH, P], F16)
            wk01 = pw.tile([P, KCH, P], F16)
            wk2d = pw.tile([P, KCH, P], F16)
            wv = pw.tile([P, KCH, 3 * DH], F16)
            bq01 = pw.tile([P, 1], F32)
            bq2d = pw.tile([P, 1], F32)
            bk01 = pw.tile([P, 1], F32)
            bk2d = pw.tile([P, 1], F32)
            bvb = pw.tile([P, 3 * DH], F32)
            wp01 = pw.tile([P, D], F16)
            wp2 = pw.tile([DH, D], F16)
            pwarm = pw.tile([P, 256], F16)
            xt = px.tile([P, KCH, S], F16)
            xtr = xt_d.ap().rearrange("(c p) s -> c p s", p=P)

            # sync queue: k01 path first (first consumer), then xt c0/c1
            nc.sync.dma_start(out=bk01, in_=bk01_d.ap())
            nc.sync.dma_start(
                out=wk01, in_=wk01_d.ap().rearrange("(c p) m -> p c m", p=P))
            nc.sync.dma_start(out=xt[:, 0, :], in_=xtr[0])
            nc.sync.dma_start(out=xt[:, 1, :], in_=xtr[1])
            nc.sync.dma_start(out=wp01, in_=wp01_d.ap())
            nc.sync.dma_start(out=wp2, in_=wp2_d.ap())
            # gpsimd (Pool) queue
            nc.gpsimd.dma_start(out=bk2d, in_=bk2d_d.ap())
            nc.gpsimd.dma_start(
                out=wk2d, in_=wk2d_d.ap().rearrange("(c p) m -> p c m", p=P))
            nc.gpsimd.dma_start(out=xt[:, 2, :], in_=xtr[2])
            nc.gpsimd.dma_start(out=xt[:, 3, :], in_=xtr[3])
            nc.gpsimd.dma_start(out=bvb, in_=bv_d.ap().to_broadcast([P, 3 * DH]))
            # scalar (ACT) queue — startup only, ACT is otherwise idle here
            nc.scalar.dma_start(out=bq01, in_=bq01_d.ap())
            nc.scalar.dma_start(
                out=wq01, in_=wq01_d.ap().rearrange("(c p) m -> p c m", p=P))
            nc.scalar.dma_start(out=xt[:, 4, :], in_=xtr[4])
            nc.scalar.dma_start(out=xt[:, 5, :], in_=xtr[5])
            nc.scalar.dma_start(
                out=wq2d, in_=wq2d_d.ap().rearrange("(c p) m -> p c m", p=P))
            nc.scalar.dma_start(
                out=wv, in_=wv_d.ap().rearrange("(c p) m -> p c m", p=P))
            nc.scalar.dma_start(out=bq2d, in_=bq2d_d.ap())

            # ---- persistent SBUF tensors ----
            k01 = pqk.tile([P, S], F16)
            k2d = pqk.tile([P, S], F16)
            q01 = pqk.tile([P, S], F16)
            q2d = pqk.tile([P, S], F16)
            v3 = pv.tile([P, NKC, 3, DH + 1], F16)
            ctn01 = pctn.tile([P, NQT, 512], F16)
            ctn2 = pctn.tile([DH, NQT, 512], F16)
            warm = pz.tile([P, 1], F32, name="warm")
            warm16 = pz.tile([P, 1], F16, name="warm16")
            # v3's ones columns via memset: the v1/v2 DMA broadcast of a
            # [1,1] DRAM scalar into this scattered region took 20-40us
            # on hardware and stalled the first context matmuls.
            nc.gpsimd.memset(v3[:, :, :, DH:DH + 1], 1.0)

            def ring(name):
                return ps.tile([P, 2, 512], F32, tag="s", bufs=3, name=name,
                               uniquify=True)

            # PE P-state warmup: ~3us of dummy matmuls on zeros ramps the
            # tensor engine to full clock while the input DMAs stream in.
            nc.vector.memset(pwarm, 0.0)
            wacc = ring("wacc")
            for i in range(12):
                nc.tensor.matmul(wacc[:, 0, 0:256], pwarm[:, 0:P],
                                 pwarm[:, 0:256], start=(i == 0),
                                 stop=(i == 11))

            # accumulate in the order xt chunks land from the 3 DMA queues
            CORDER = [0, 2, 4, 1, 3, 5]

            def stream_chunk(dst, w, bias, qt, name):
                # one 512-wide slab of a QKV output stream
                acc = ring(name)
                for ci, c in enumerate(CORDER):
                    nc.tensor.matmul(
                        acc[:, 0, :], w[:, c, :],
                        xt[:, c, qt * 512:(qt + 1) * 512],
                        start=(ci == 0), stop=(ci == KCH - 1))
                nc.vector.tensor_scalar_add(
                    out=dst[:, qt * 512:(qt + 1) * 512], in0=acc[:, 0, :],
                    scalar1=bias)

            def v_group(sc):
                acc = ring(f"v{sc}")
                for c in range(KCH):
                    nc.tensor.matmul(
                        acc[:, 0, 0:3 * DH], xt[:, c, sc * P:(sc + 1) * P],
                        wv[:, c, :], start=(c == 0), stop=(c == KCH - 1))
                for h in range(3):
                    nc.vector.tensor_add(
                        v3[:, sc, h, 0:DH], acc[:, 0, h * DH:(h + 1) * DH],
                        bvb[:, h * DH:(h + 1) * DH])

            # ---- round definitions ----
            # one round = 2 paired score matmuls -> exp -> 2 ctx matmuls.
            # p01 rounds: heads 0,1 at key chunk c.  h2 rounds: head 2 at
            # key chunks (2rr, 2rr+1) via the duplicated k2d/q2d halves.
            ct = {}

            def make_p01(qt, c):
                def scores(slot):
                    q0 = qt * 512
                    nc.tensor.matmul(
                        slot[:, 0, :],
                        k01[0:DH, c * P:(c + 1) * P],
                        q01[0:DH, q0:q0 + 512], start=True, stop=True)
                    nc.tensor.matmul(
                        slot[:, 1, :],
                        k01[DH:P, c * P:(c + 1) * P],
                        q01[DH:P, q0:q0 + 512], start=True, stop=True)

                def ctx(pt):
                    if c == 0:
                        ct[(qt, 0)] = ps.tile([DH + 1, 512], F32, tag="ct",
                                              bufs=2, name=f"ct0_{qt}",
                                              uniquify=True)
                        ct[(qt, 1)] = ps.tile([DH + 1, 512], F32, tag="ct",
                                              bufs=2, name=f"ct1_{qt}",
                                              uniquify=True)
                    for h in range(2):
                        nc.tensor.matmul(
                            ct[(qt, h)], v3[:, c, h, :],
                            pt[:, h, :], start=(c == 0), stop=(c == NKC - 1))
                return scores, ctx

            def make_h2(qt, rr):
                c = 2 * rr

                def scores(slot):
                    q0 = qt * 512
                    nc.tensor.matmul(
                        slot[:, 0, :],
                        k2d[0:DH, c * P:(c + 1) * P],
                        q2d[0:DH, q0:q0 + 512], start=True, stop=True)
                    nc.tensor.matmul(
                        slot[:, 1, :],
                        k2d[DH:P, (c + 1) * P:(c + 2) * P],
                        q2d[DH:P, q0:q0 + 512], start=True, stop=True)

                def ctx(pt):
                    if rr == 0:
                        ct[(qt, 2)] = ps.tile([DH + 1, 512], F32, tag="ct",
                                              bufs=2, name=f"ct2_{qt}",
                                              uniquify=True)
                    for j in range(2):
                        nc.tensor.matmul(
                            ct[(qt, 2)], v3[:, c + j, 2, :],
                            pt[:, j, :], start=(rr == 0 and j == 0),
                            stop=(rr == NKC // 2 - 1 and j == 1))
                return scores, ctx

            pend = {}

            def norm_a(qt, heads, direct=False):
                # DVE copies free the ct PSUM slots.  1/Z: the Z rows are
                # bounced to DRAM and re-read spread over 128 partitions,
                # so the reciprocal costs ~4 elements/partition instead
                # of a 3.3us single-partition crawl that head-of-line
                # blocks the in-order DVE queue.  Broadcast back via the
                # DRAM to_broadcast read.  No PE or PSUM involvement.
                ctus = []
                for h in heads:
                    ctu = pz.tile([DH + 1, 512], F32R, tag="ctu", bufs=3,
                                  name=f"cu{h}{qt}", uniquify=True)
                    nc.vector.tensor_copy(ctu, ct[(qt, h)])
                    ctus.append(ctu)
                n = len(heads)
                zs = []
                if direct:
                    # kernel tail: latency beats throughput, DVE is idle
                    for j, h in enumerate(heads):
                        zrow = ctus[j][DH:DH + 1, :].bitcast(F32)
                        nc.vector.reciprocal(out=zrow, in_=zrow)
                        zd = pdram.tile([1, 512], F32, tag="zd",
                                        name=f"zd{h}{qt}", uniquify=True)
                        nc.sync.dma_start(out=zd, in_=zrow)
                        rp = pz.tile([DH, 512], F32, tag="rp", bufs=3,
                                     name=f"rp{h}{qt}", uniquify=True)
                        nc.gpsimd.dma_start(
                            out=rp, in_=zd.to_broadcast([DH, 512]))
                        zs.append((h, ctus[j], rp))
                else:
                    zg = pz.tile([P, 4 * n], F32, tag="zg", bufs=2,
                                 name=f"zg{qt}{heads[0]}", uniquify=True)
                    zd = pdram.tile([P, 4 * n], F32, tag="zd",
                                    name=f"zd{qt}{heads[0]}", uniquify=True)
                    zflat = zd.rearrange("p f -> (p f)")
                    for j, h in enumerate(heads):
                        nc.sync.dma_start(
                            out=zflat[j * 512:(j + 1) * 512].rearrange(
                                "(o q) -> o q", o=1),
                            in_=ctus[j][DH:DH + 1, :].bitcast(F32))
                    nc.sync.dma_start(out=zg, in_=zd)
                    nc.vector.reciprocal(out=zg, in_=zg)
                    zd2 = pdram.tile([P, 4 * n], F32, tag="zd2",
                                     name=f"ze{qt}{heads[0]}", uniquify=True)
                    nc.sync.dma_start(out=zd2, in_=zg)
                    zflat2 = zd2.rearrange("p f -> (p f)")
                    for j, h in enumerate(heads):
                        rp = pz.tile([DH, 512], F32, tag="rp", bufs=3,
                                     name=f"rp{h}{qt}", uniquify=True)
                        nc.gpsimd.dma_start(
                            out=rp,
                            in_=zflat2[j * 512:(j + 1) * 512].rearrange(
                                "(o q) -> o q", o=1).to_broadcast([DH, 512]))
                        zs.append((h, ctus[j], rp))
                pend[qt, heads[0]] = zs

            def norm_b(qt, h0):
                for h, ctu, rp in pend.pop((qt, h0)):
                    if h == 0:
                        dst = ctn01[0:DH, qt, :]
                    elif h == 2:
                        dst = ctn2[:, qt, :]
                    else:
                        dst = pz.tile([DH, 512], F16, tag="c1t", bufs=2,
                                      name=f"c1t{qt}", uniquify=True)
                    nc.vector.tensor_mul(
                        dst, ctu[0:DH, :].bitcast(F32), rp)
                    if h == 1:
                        # partition shift 0:64 -> 64:128 via DRAM bounce
                        c1d = pdram.tile([DH, 512], F16, tag="c1d",
                                         name=f"c1d{qt}", uniquify=True)
                        nc.sync.dma_start(out=c1d, in_=dst)
                        nc.sync.dma_start(out=ctn01[DH:P, qt, :], in_=c1d)

            def proj_st(qt, st):
                pp = ring(f"pp{qt}{st}")
                lhs01 = ctn01[:, qt, st * P:(st + 1) * P]
                lhs2 = ctn2[:, qt, st * P:(st + 1) * P]
                for r0, r1, reg in ((0, 512, 0), (512, D, 1)):
                    nc.tensor.matmul(pp[:, reg, 0:r1 - r0], lhs01,
                                     wp01[:, r0:r1], start=True, stop=False)
                    nc.tensor.matmul(pp[:, reg, 0:r1 - r0], lhs2,
                                     wp2[:, r0:r1], start=False, stop=True)
                stage = pout.tile([P, D], F32, tag="stage",
                                  name=f"st{qt}{st}", uniquify=True)
                nc.vector.tensor_copy(stage[:, 0:512], pp[:, 0, :])
                nc.vector.tensor_copy(stage[:, 512:D], pp[:, 1, 0:256])
                r0 = qt * 512 + st * P
                nc.gpsimd.dma_start(out=out_d.ap()[r0:r0 + P, :], in_=stage)

            # ---- build the global round list with post-work ----
            rounds = []
            posts = {}
            for qt in range(NQT):
                base = qt * 24
                for c in range(NKC):
                    rounds.append(make_p01(qt, c))
                for rr in range(NKC // 2):
                    rounds.append(make_h2(qt, rr))
                posts.setdefault(base + 15, []).append(
                    lambda qt=qt: norm_a(qt, [0, 1]))
                posts.setdefault(base + 21, []).append(
                    lambda qt=qt: norm_b(qt, 0))
                posts.setdefault(base + 23, []).append(
                    lambda qt=qt: norm_a(qt, [2], direct=(qt == NQT - 1)))
                if qt < NQT - 1:
                    posts.setdefault(base + 29, []).append(
                        lambda qt=qt: norm_b(qt, 2))
                if qt < NQT - 1:
                    # spread proj st-chunks well into the next qt's rounds
                    # so they never wait on the normalize chain from the
                    # in-order PE stream
                    for st in range(4):
                        posts.setdefault(base + 33 + 4 * st, []).append(
                            lambda qt=qt, st=st: proj_st(qt, st))
            # qt0 fill-in: V groups and remaining QKV streams
            posts[0] = [lambda: v_group(2), lambda: v_group(3)]
            posts[1] = [lambda: stream_chunk(q2d, wq2d, bq2d, 0, "q2d0")]
            posts[2] = [lambda: v_group(4), lambda: v_group(5)]
            posts[3] = [lambda: stream_chunk(k2d, wk2d, bk2d, 0, "k2d0")]
            posts[4] = [lambda: v_group(6), lambda: v_group(7)]
            posts[5] = [lambda: stream_chunk(k2d, wk2d, bk2d, 1, "k2d1")]
            posts[6] = [lambda: v_group(8), lambda: v_group(9)]
            posts[7] = [lambda: stream_chunk(k2d, wk2d, bk2d, 2, "k2d2")]
            posts[8] = [lambda: v_group(10), lambda: v_group(11)]
            posts[9] = [lambda: stream_chunk(k2d, wk2d, bk2d, 3, "k2d3")]
            posts[10] = [lambda: v_group(12), lambda: v_group(13)]
            posts[11] = [lambda: stream_chunk(q01, wq01, bq01, 1, "q011")]
            posts[12] = [lambda: v_group(14), lambda: v_group(15)]
            posts[13] = [lambda: stream_chunk(q2d, wq2d, bq2d, 1, "q2d1")]
            # later qt fill-in: next qt's q streams
            for qt in (1, 2):
                posts.setdefault(qt * 24 + 3, []).append(
                    lambda qt=qt: stream_chunk(q01, wq01, bq01, qt + 1,
                                               f"q01{qt + 1}"))
                posts.setdefault(qt * 24 + 9, []).append(
                    lambda qt=qt: stream_chunk(q2d, wq2d, bq2d, qt + 1,
                                               f"q2d{qt + 1}"))

            # k01 slabs 1-3 land mid-stream, just before the first scores
            # round that reads them (round 4s reads slab s)
            pres = {2: [lambda: stream_chunk(k01, wk01, bk01, 1, "k011")],
                    6: [lambda: stream_chunk(k01, wk01, bk01, 2, "k012")],
                    10: [lambda: stream_chunk(k01, wk01, bk01, 3, "k013")]}

            # ---- prologue: only what rounds 0-3 need ----
            stream_chunk(k01, wk01, bk01, 0, "k010")
            stream_chunk(q01, wq01, bq01, 0, "q010")
            v_group(0)
            v_group(1)
            # warm the ACT exp table so round 0 doesn't pay the load
            nc.vector.memset(warm, 1.0)
            nc.scalar.activation(warm16, warm, EXP, scale=0.125)

            # ---- software-pipelined main loop ----
            NR = len(rounds)
            slots = {}
            pts = {}
            for i in range(NR + 2):
                for fn in pres.get(i, ()):
                    fn()
                if i < NR:
                    slots[i] = ring(f"r{i}")
                    rounds[i][0](slots[i])
                if i >= 1 and i - 1 < NR:
                    j = i - 1
                    pts[j] = ppt.tile([P, 2, 512], F16, tag="pt", bufs=6,
                                      name=f"pt{j}", uniquify=True)
                    nc.scalar.activation(pts[j], slots[j], EXP, scale=0.125)
                if i >= 2:
                    j = i - 2
                    rounds[j][1](pts[j])
                    del slots[j], pts[j]
                    for fn in posts.get(j, ()):
                        fn()

            # qt3 normalize-b + projection tail (no later rounds)
            norm_b(NQT - 1, 2)
            for st in range(4):
                proj_st(NQT - 1, st)

    nc.compile()
    return nc


def _get_nc():
    if "nc" not in _CACHE:
        _CACHE["nc"] = _build()
    return _CACHE["nc"]


def kernel(x, attention_mask, w_qkv, b_qkv, w_proj, b_proj, _trace=False):
    from concourse.bass_utils import run_bass_kernel_spmd

    x = np.asarray(x, dtype=np.float32)
    w_qkv = np.asarray(w_qkv, dtype=np.float32)
    b_qkv = np.asarray(b_qkv, dtype=np.float32)
    w_proj = np.asarray(w_proj, dtype=np.float32)
    b_proj = np.asarray(b_proj, dtype=np.float32)

    in_maps = []
    for core in range(NCORES):
        b, g = divmod(core, 4)
        base = g * 3 * DH
        wq2 = w_qkv[:, base + 2 * DH:base + 3 * DH]
        wk2 = w_qkv[:, D + base + 2 * DH:D + base + 3 * DH]
        bq2 = b_qkv[base + 2 * DH:base + 3 * DH]
        bk2 = b_qkv[D + base + 2 * DH:D + base + 3 * DH]
        f16 = np.float16
        in_maps.append({
            "xt": np.ascontiguousarray(x[b].T.astype(f16)),
            "wq01": np.ascontiguousarray(
                w_qkv[:, base:base + 2 * DH].astype(f16)),
            "wq2d": np.ascontiguousarray(
                np.concatenate([wq2, wq2], axis=1).astype(f16)),
            "wk01": np.ascontiguousarray(
                w_qkv[:, D + base:D + base + 2 * DH].astype(f16)),
            "wk2d": np.ascontiguousarray(
                np.concatenate([wk2, wk2], axis=1).astype(f16)),
            "wv": np.ascontiguousarray(
                w_qkv[:, 2 * D + base:2 * D + base + 3 * DH].astype(f16)),
            "bq01": np.ascontiguousarray(b_qkv[base:base + 2 * DH].reshape(P, 1)),
            "bq2d": np.ascontiguousarray(
                np.concatenate([bq2, bq2]).reshape(P, 1)),
            "bk01": np.ascontiguousarray(
                b_qkv[D + base:D + base + 2 * DH].reshape(P, 1)),
            "bk2d": np.ascontiguousarray(
                np.concatenate([bk2, bk2]).reshape(P, 1)),
            "bv": np.ascontiguousarray(
                b_qkv[2 * D + base:2 * D + base + 3 * DH].reshape(1, 3 * DH)),
            "wp01": np.ascontiguousarray(
                w_proj[base:base + 2 * DH, :].astype(f16)),
            "wp2": np.ascontiguousarray(
                w_proj[base + 2 * DH:base + 3 * DH, :].astype(f16)),
        })

    nc = _get_nc()
    # Warmup execution: the very first run after NEFF load can race the
    # ACT function-table load, corrupting a few exp results. Tables are
    # resident afterwards, so the second run is clean — return that one.
    run_bass_kernel_spmd(nc, in_maps, list(range(NCORES)), trace=False)
    res = run_bass_kernel_spmd(nc, in_maps, list(range(NCORES)), trace=_trace)
    if _trace:
        _CACHE["last_result"] = res

    out = np.zeros((B, S, D), dtype=np.float32)
    for core in range(NCORES):
        b = core // 4
        out[b] += res.results[core]["out"]
    out += b_proj[None, None, :]
    return out
